# revision 1
# baseline (speedup 1.0000x reference)
"""LlamaMlpWithLora on 8 Trainium2 NeuronCores — token-parallel version.

Pure data-parallel over tokens: each core owns T/8 = 512 tokens and runs the
FULL MLP (I = 11008 = 86*128, zero padding) for them; the host just
concatenates the 8 output slices — no reduction. Weights are streamed from
HBM exactly once per core; x (4 MB) and the gate*up activation (11 MB)
stay resident in SBUF. All matmuls run in bf16 with fp32 PSUM accumulation.

Per-core PE work: 32 (loraA gate/up) + 86*33*2 (gate/up + loraB folds)
+ 86 (down loraA) + 8*4*(86+1) (down + loraB folds) = 8578 matmuls of
512 columns ~= 1.83 ms at 2.4 GHz.
"""

import contextlib
import sys

sys.path.insert(0, "/opt/trn_rl_repo")

import numpy as np
import ml_dtypes

T, H, I, R, A = 4096, 4096, 11008, 16, 4
NC_CORES = 8
TPC = T // NC_CORES      # 512 tokens per core
NIT = I // 128           # 86 i-tiles (exact, no padding)
NKT = H // 128           # 32 contraction tiles over hidden dim
NHB = H // 512           # 8 output-column blocks of 512
NT4 = TPC // 128         # 4 token sub-tiles of 128

_cached = {}             # reps -> compiled program (compile once per process)


def _build_program(reps=1):
    import concourse.bass as bass
    import concourse.tile as tile
    from concourse import bacc, mybir

    bf = mybir.dt.bfloat16
    f32 = mybir.dt.float32
    mult = mybir.AluOpType.mult
    ds = bass.ds
    silu = mybir.ActivationFunctionType.Silu

    nc = bacc.Bacc("TRN2", target_bir_lowering=False, debug=False,
                   num_devices=NC_CORES)

    # DRAM inputs, pre-tiled on host so every DMA slice is contiguous.
    xt = nc.dram_tensor("xt", [128, NKT, TPC], bf, kind="ExternalInput")
    gw = nc.dram_tensor("gw", [NIT * 128, NKT, 128], bf, kind="ExternalInput")
    uw = nc.dram_tensor("uw", [NIT * 128, NKT, 128], bf, kind="ExternalInput")
    dw = nc.dram_tensor("dw", [NHB * 128, NIT, 512], bf, kind="ExternalInput")
    wagu = nc.dram_tensor("wagu", [128, NKT, 128], bf, kind="ExternalInput")
    gub = nc.dram_tensor("gub", [128, NIT, 128], bf, kind="ExternalInput")
    dwa = nc.dram_tensor("dwa", [128, NIT, 64], bf, kind="ExternalInput")
    dwb = nc.dram_tensor("dwb", [64, NHB, 512], bf, kind="ExternalInput")
    mask = nc.dram_tensor("mask", [64, TPC], f32, kind="ExternalInput")
    out = nc.dram_tensor("out", [TPC, H], f32, kind="ExternalOutput")

    with tile.TileContext(nc) as tc:
        with (
            tc.tile_pool(name="const", bufs=1) as cpool,
            tc.tile_pool(name="act", bufs=1) as actpool,
            tc.tile_pool(name="xa", bufs=1) as xapool,
        ):
            wagu_sb = cpool.tile([128, NKT, 128], bf)
            nc.sync.dma_start(wagu_sb[:], wagu[:])
            mask_sb = cpool.tile([64, TPC], f32)
            nc.sync.dma_start(mask_sb[:], mask[:])
            # late-phase constants ride the scalar queue so they don't
            # delay x/gw on the sync queue at kernel start
            gub_sb = cpool.tile([128, NIT, 128], bf)
            nc.scalar.dma_start(gub_sb[:], gub[:])
            dwa_sb = cpool.tile([128, NIT, 64], bf)
            nc.scalar.dma_start(dwa_sb[:], dwa[:])
            dwb_sb = cpool.tile([64, NHB, 512], bf)
            nc.scalar.dma_start(dwb_sb[:], dwb[:])

            # reps>1 repeats the computation on-device (timing builds only)
            loop_ctx = (tc.For_i(0, reps, 1) if reps > 1
                        else contextlib.nullcontext())
            with loop_ctx:
              act_sb = actpool.tile([128, NIT, TPC], bf, tag="act")
              # gate LoRA-A activations on partitions 0:64, up on 64:128 so
              # the fold matmuls see matching base partitions with gub_sb
              xagu = xapool.tile([128, TPC], bf, tag="xagu")
              xad = xapool.tile([64, TPC], bf, tag="xad")

              # ---- phase 1: gate/up projections + silu*up ----
              with (
                  tc.tile_pool(name="x", bufs=1) as xpool,
                  tc.tile_pool(name="w", bufs=3) as wpool,
                  tc.tile_pool(name="tmp", bufs=2) as tmppool,
                  tc.tile_pool(name="psg", bufs=2, space="PSUM") as psg,
                  tc.tile_pool(name="psu", bufs=2, space="PSUM") as psu,
                  tc.tile_pool(name="psxa", bufs=1, space="PSUM") as psxa,
              ):
                xt_sb = xpool.tile([128, NKT, TPC], bf, tag="x")
                for xc in range(4):
                    nc.sync.dma_start(xt_sb[:, ds(xc * 8, 8), :],
                                      xt[:, ds(xc * 8, 8), :])

                # LoRA A projection for gate (rows 0:64) and up (rows 64:128)
                pxa = psxa.tile([128, TPC], f32, tag="pxa")
                for k in range(NKT):
                    nc.tensor.matmul(pxa[:], wagu_sb[:, k, :], xt_sb[:, k, :],
                                     start=(k == 0), stop=(k == NKT - 1))
                nc.vector.tensor_tensor(xagu[0:64, :], pxa[0:64, :],
                                        mask_sb[:], mult)
                nc.vector.tensor_tensor(xagu[64:128, :], pxa[64:128, :],
                                        mask_sb[:], mult)

                for io in range(NIT):
                    gw_sb = wpool.tile([128, NKT, 128], bf, tag="w")
                    nc.sync.dma_start(gw_sb[:], gw[ds(io * 128, 128), :, :])
                    pg = psg.tile([128, TPC], f32, tag="pg")
                    for k in range(NKT):
                        nc.tensor.matmul(pg[:], gw_sb[:, k, :], xt_sb[:, k, :],
                                         start=(k == 0), stop=False)
                    nc.tensor.matmul(pg[:], gub_sb[0:64, io, :],
                                     xagu[0:64, :], start=False, stop=True)

                    uw_sb = wpool.tile([128, NKT, 128], bf, tag="w")
                    nc.gpsimd.dma_start(uw_sb[:],
                                        uw[ds(io * 128, 128), :, :])
                    pu = psu.tile([128, TPC], f32, tag="pu")
                    for k in range(NKT):
                        nc.tensor.matmul(pu[:], uw_sb[:, k, :], xt_sb[:, k, :],
                                         start=(k == 0), stop=False)
                    nc.tensor.matmul(pu[:], gub_sb[64:128, io, :],
                                     xagu[64:128, :], start=False, stop=True)

                    tmp = tmppool.tile([128, TPC], f32, tag="tmp")
                    nc.scalar.activation(tmp[:], pg[:], silu)
                    nc.vector.tensor_tensor(act_sb[:, io, :], tmp[:], pu[:],
                                            mult)

              # ---- phase 2a: down LoRA A projection ----
              with tc.tile_pool(name="psxad", bufs=1, space="PSUM") as psxad:
                pxad = psxad.tile([64, TPC], f32, tag="pxad")
                for io in range(NIT):
                    nc.tensor.matmul(pxad[:], dwa_sb[:, io, :],
                                     act_sb[:, io, :],
                                     start=(io == 0), stop=(io == NIT - 1))
                nc.vector.tensor_tensor(xad[:], pxad[:], mask_sb[:], mult)

              # ---- phase 2b: down projection ----
              with (
                  tc.tile_pool(name="dwp", bufs=8) as dwpool,
                  tc.tile_pool(name="o", bufs=4) as opool,
                  tc.tile_pool(name="psd", bufs=8, space="PSUM") as psd,
              ):
                for hb in range(NHB):
                    pd = [psd.tile([128, 512], f32, tag="pd",
                                   name=f"pd{hb}_{t4}")
                          for t4 in range(NT4)]
                    for k2 in range(NIT // 2):
                        dw_sb = dwpool.tile([128, 2, 512], bf, tag="dw")
                        nc.scalar.dma_start(
                            dw_sb[:], dw[ds(hb * 128, 128),
                                         ds(k2 * 2, 2), :])
                        for kk in range(2):
                            k = k2 * 2 + kk
                            for t4 in range(NT4):
                                nc.tensor.matmul(
                                    pd[t4],
                                    act_sb[:, k, ds(t4 * 128, 128)],
                                    dw_sb[:, kk, :],
                                    start=(k == 0), stop=False)
                    for t4 in range(NT4):
                        nc.tensor.matmul(pd[t4], xad[:, ds(t4 * 128, 128)],
                                         dwb_sb[:, hb, :],
                                         start=False, stop=True)
                        o_sb = opool.tile([128, 512], f32, tag="o")
                        nc.vector.tensor_copy(o_sb[:], pd[t4])
                        nc.gpsimd.dma_start(
                            out[ds(t4 * 128, 128), ds(hb * 512, 512)],
                            o_sb[:])

    nc.compile()
    return nc


def _host_prep(x, gate_w, up_w, down_w, gate_wa, gate_wb, up_wa, up_wb,
               down_wa, down_wb, seg_ids):
    """Transpose/tile all operands; returns per-core input maps."""
    bf16 = ml_dtypes.bfloat16

    # x^T per core: [128, NKT, TPC]
    xT = np.ascontiguousarray(x.T)                                  # [H, T]
    xt_full = xT.reshape(NKT, 128, NC_CORES, TPC).transpose(2, 1, 0, 3)
    xt_c = [np.ascontiguousarray(xt_full[c].astype(bf16))
            for c in range(NC_CORES)]

    def gu_tiles(w):  # w: [I, H] -> [NIT*128, NKT, 128] (lhsT tiles)
        t = w.T.reshape(NKT, 128, NIT, 128).transpose(2, 1, 0, 3)
        return np.ascontiguousarray(
            t.reshape(NIT * 128, NKT, 128).astype(bf16))

    gw_t = gu_tiles(gate_w)
    uw_t = gu_tiles(up_w)

    # down_w [H, I] -> down_wT [I, H] -> [NHB*128, NIT, 512]
    t = down_w.T.reshape(NIT, 128, NHB, 512).transpose(2, 1, 0, 3)
    dw_t = np.ascontiguousarray(t.reshape(NHB * 128, NIT, 512).astype(bf16))

    # LoRA A for gate+up, concatenated: [H, 128] -> [128, NKT, 128]
    wa = np.concatenate([gate_wa.transpose(1, 0, 2).reshape(H, A * R),
                         up_wa.transpose(1, 0, 2).reshape(H, A * R)], axis=1)
    wagu_t = np.ascontiguousarray(
        wa.reshape(NKT, 128, 128).transpose(1, 0, 2).astype(bf16))

    # LoRA B for gate (rows 0:64) and up (rows 64:128): [128, NIT, 128]
    gub_t = np.ascontiguousarray(np.concatenate(
        [gate_wb.reshape(A * R, NIT, 128), up_wb.reshape(A * R, NIT, 128)],
        axis=0).astype(bf16))

    # down LoRA A [A, I, R] -> [I, 64] -> [128, NIT, 64]
    dwa_f = down_wa.transpose(1, 0, 2).reshape(I, A * R)
    dwa_t = np.ascontiguousarray(
        dwa_f.reshape(NIT, 128, 64).transpose(1, 0, 2).astype(bf16))

    # down LoRA B [A, R, H] -> [64, NHB, 512] (replicated)
    dwb_t = np.ascontiguousarray(
        down_wb.reshape(A * R, NHB, 512).astype(bf16))

    # adapter mask rows a*R+r, per-core token slice: [64, TPC] fp32
    m = (seg_ids[None, :] == np.arange(A, dtype=seg_ids.dtype)[:, None])
    mask_full = np.repeat(m, R, axis=0).astype(np.float32)          # [64, T]
    mask_c = [np.ascontiguousarray(mask_full[:, c * TPC:(c + 1) * TPC])
              for c in range(NC_CORES)]

    in_maps = []
    for c in range(NC_CORES):
        in_maps.append({
            "xt": xt_c[c], "gw": gw_t, "uw": uw_t, "dw": dw_t,
            "wagu": wagu_t, "gub": gub_t, "dwa": dwa_t, "dwb": dwb_t,
            "mask": mask_c[c],
        })
    return in_maps


def get_program(reps=1):
    if reps not in _cached:
        _cached[reps] = _build_program(reps)
    return _cached[reps]


def kernel(x, gate_w, up_w, down_w, gate_wa, gate_wb, up_wa, up_wb,
           down_wa, down_wb, seg_ids):
    from concourse.bass_utils import run_bass_kernel_spmd

    x, gate_w, up_w, down_w = (np.asarray(a, np.float32) for a in
                               (x, gate_w, up_w, down_w))
    gate_wa, gate_wb, up_wa, up_wb, down_wa, down_wb = (
        np.asarray(a, np.float32) for a in
        (gate_wa, gate_wb, up_wa, up_wb, down_wa, down_wb))
    seg_ids = np.asarray(seg_ids, np.int32)

    nc = get_program()
    in_maps = _host_prep(x, gate_w, up_w, down_w, gate_wa, gate_wb,
                         up_wa, up_wb, down_wa, down_wb, seg_ids)
    res = run_bass_kernel_spmd(nc, in_maps, core_ids=list(range(NC_CORES)))
    return np.concatenate([res.results[c]["out"] for c in range(NC_CORES)],
                          axis=0)

